# revision 18
# baseline (speedup 1.0000x reference)
"""GCC-PHAT kernel for Trainium2: x[64,12,4096] -> gcc[64,12,12,51].

Pipeline per core (8 batches, 96 signals):
  rfft(4096) as 2-stage matmul FFT (256x16 radix, f = u + 256 s),
  phase extraction (atan2), all-pair phase differences, cos/sin via ACT,
  projection onto 51 lags as accumulated matmuls.
Data parallel over 8 NeuronCores (8 batches each).

Dispatch: input shipped as fp16 (halves axon-tunnel bytes), output fp16;
the shard_map jit is built once and cached so repeat calls skip
retrace/relower, and transfers pipeline with execution.
"""
import sys
sys.path.insert(0, "/opt/trn_rl_repo")
import numpy as np
import concourse.mybir as mybir
import ml_dtypes
from concourse import bass
from concourse.tile import TileContext

F32 = mybir.dt.float32
F32R = mybir.dt.float32r
F16 = mybir.dt.float16
I8 = mybir.dt.int8
BF16 = mybir.dt.bfloat16
AF = mybir.ActivationFunctionType
ALU = mybir.AluOpType
PI = float(np.pi)

K = 4096
Q = 256
R = 16
S = 16
NSIG = 96
B = 8
TAU_MAX = 25
NLAG = 51
LAGS = np.concatenate([np.arange(TAU_MAX + 1), np.arange(-TAU_MAX, 0)])


def _trunc22(a):
    """Pre-truncate fp32 to fp22 (matmul fp32r input precision)."""
    b = np.ascontiguousarray(a, np.float32)
    u = b.view(np.uint32).copy()
    u &= np.uint32(0xFFFFFC00)
    return u.view(np.float32)


def _constants():
    c = {}
    q = np.arange(Q)
    u = np.arange(Q)
    ang = 2 * np.pi * np.outer(q, u) / Q
    c["CA"] = np.cos(ang).astype(np.float16)
    c["SA"] = (-np.sin(ang)).astype(np.float16)
    r = np.arange(R)
    tang = 2 * np.pi * np.outer(np.arange(Q), r) / K       # [u(256), r]
    # broadcast over 24-sig block: [256, 24, 16]
    c["T_re"] = np.broadcast_to(
        np.cos(tang)[:, None, :], (Q, 24, R)).reshape(Q, 384).astype(np.float32)
    c["T_im"] = np.broadcast_to(
        (-np.sin(tang))[:, None, :], (Q, 24, R)).reshape(Q, 384).astype(np.float32)
    s = np.arange(S)
    bang = 2 * np.pi * np.outer(r, s) / S
    vre = np.zeros((128, 128), np.float32)
    vim = np.zeros((128, 128), np.float32)
    for sig in range(8):
        vre[sig * 16:sig * 16 + 16, sig * 16:sig * 16 + 16] = np.cos(bang)
        vim[sig * 16:sig * 16 + 16, sig * 16:sig * 16 + 16] = -np.sin(bang)
    c["V_re"] = _trunc22(vre)
    c["V_im"] = _trunc22(vim)
    c["V_nim"] = _trunc22(-vim)
    w = np.zeros(K // 2 + 1)
    w[1:K // 2] = 2.0 / K
    w[0] = 1.0 / K
    w[K // 2] = 1.0 / K
    EC = np.zeros((2, 9, 128, NLAG), np.float32)
    ES = np.zeros((2, 9, 128, NLAG), np.float32)
    for uc in range(2):
        for s9 in range(9):
            f = 128 * uc + np.arange(128) + 256 * s9
            valid = f <= K // 2
            wf = np.where(valid, w[np.minimum(f, K // 2)], 0.0)
            th = 2 * np.pi * np.outer(f, LAGS) / K
            EC[uc, s9] = -wf[:, None] * np.cos(th)
            ES[uc, s9] = wf[:, None] * np.sin(th)
    c["EC"] = (2.0 * EC).astype(ml_dtypes.bfloat16)
    c["ES"] = ES.astype(ml_dtypes.bfloat16)
    c["IDT"] = np.eye(128, dtype=np.float32)
    return c


def _split_excess_waits(nc, limit=1):
    n_split = 0
    for f in nc.m.functions:
        for blk in f.blocks:
            i = 0
            while i < len(blk.instructions):
                inst = blk.instructions[i]
                si = inst.sync_info
                if si is not None and len(si.on_wait) > limit:
                    waits = list(si.on_wait)
                    si.on_wait = waits[:limit]
                    excess = waits[limit:]
                    for j in range(0, len(excess), limit):
                        nop = mybir.InstNoOp(
                            name=f"waitsplit_{n_split}", ins=[], outs=[])
                        n_split += 1
                        nop.engine = inst.engine
                        nop.sync_info = mybir.SyncInfo(
                            on_wait=excess[j:j + limit], on_update=[])
                        nc.register_instruction(nop)
                        blk.instructions.insert(i, nop)
                        i += 1
                i += 1
    return n_split


def build_nc():
    c = _constants()
    nc = bass.Bass()

    def reg_const(value):
        t = nc.alloc_sbuf_tensor(f"cap-{value}", [128, 1], F32)
        nc.gpsimd.memset(t.ap(), value)
        nc.const_aps.aps[(F32, value)] = t.ap()

    for v in (-PI, -PI / 2, PI / 2, PI, 2 * PI, -2 * PI):
        reg_const(float(v))

    # int8 input: GCC-PHAT is scale-invariant (PHAT normalizes each bin to
    # unit magnitude), so the host-side quantization scale cancels and no
    # dequant is needed on device.
    x_h = nc.declare_dram_parameter("x", [B, 12, K], I8, isOutput=False)
    g_h = nc.declare_dram_parameter("g", [B, 144, NLAG], F16, isOutput=True)

    ca_h = nc.inline_tensor(c["CA"], "ca")          # [256, 256] f16
    sa_h = nc.inline_tensor(c["SA"], "sa")
    tre_h = nc.inline_tensor(c["T_re"], "tre")      # [256, 384]
    tim_h = nc.inline_tensor(c["T_im"], "tim")
    vre_h = nc.inline_tensor(c["V_re"], "vre")
    vim_h = nc.inline_tensor(c["V_im"], "vim")
    vnim_h = nc.inline_tensor(c["V_nim"], "vnim")
    ec_h = nc.inline_tensor(c["EC"], "ec")          # [2, 9, 128, 51]
    es_h = nc.inline_tensor(c["ES"], "es")
    idt_h = nc.inline_tensor(c["IDT"], "idt")

    xv = x_h[:].rearrange("b n (q r) -> q (b n) r", q=Q, r=R)   # [256, 96, 16]

    with TileContext(nc, pool_alloc_mode="queue") as tc:
        with tc.tile_pool(name="consts", bufs=1) as cpool:
            # stage-A DFT chunks [qc][uc]
            cs_t = {}
            for (nm, h) in (("c", ca_h), ("s", sa_h)):
                for qc in range(2):
                    for uc in range(2):
                        t = cpool.tile([128, 128], F16, tag=f"cs{nm}{qc}{uc}", name=f"cs{nm}{qc}{uc}")
                        [nc.scalar, nc.gpsimd][(qc + uc) % 2].dma_start(
                            out=t[:],
                            in_=h[128 * qc:128 * qc + 128,
                                  128 * uc:128 * uc + 128])
                        cs_t[(nm, qc, uc)] = t
            tw_t = {}
            for (nm, h) in (("re", tre_h), ("im", tim_h)):
                for uc in range(2):
                    t = cpool.tile([128, 384], F32, tag=f"tw{nm}{uc}", name=f"tw{nm}{uc}")
                    [nc.scalar, nc.gpsimd][uc % 2].dma_start(
                        out=t[:], in_=h[128 * uc:128 * uc + 128, :])
                    tw_t[(nm, uc)] = t
            v_t = {}
            for nm, h in (("re", vre_h), ("im", vim_h), ("nim", vnim_h)):
                t = cpool.tile([128, 128], F32R, tag=f"v{nm}", name=f"v{nm}")
                nc.scalar.dma_start(out=t[:], in_=h[:].bitcast(F32R))
                v_t[nm] = t
            ec_t = cpool.tile([128, 2, 9, NLAG], BF16, tag="ec")
            nc.gpsimd.dma_start(
                out=ec_t[:], in_=ec_h[:].rearrange("a s u t -> u a s t"))
            es_t = cpool.tile([128, 2, 9, NLAG], BF16, tag="es")
            nc.scalar.dma_start(
                out=es_t[:], in_=es_h[:].rearrange("a s u t -> u a s t"))
            idt_t = cpool.tile([128, 128], F32, tag="idt")
            nc.sync.dma_start(out=idt_t[:], in_=idt_h[:])

            with tc.tile_pool(name="work", bufs=1) as wpool:
                # persistent big tiles
                zp = {}
                for nm in ("re", "im"):
                    for uc in range(2):
                        zp[(nm, uc)] = wpool.tile([128, 1536], F32, tag=f"zp{nm}{uc}", name=f"zp{nm}{uc}")
                xfm = {}
                for nm in ("re", "im"):
                    for uc in range(2):
                        xfm[(nm, uc)] = wpool.tile([128, 864], F32, tag=f"xfm{nm}{uc}", name=f"xfm{nm}{uc}")

                # ---- Phase 1: stage-A matmuls + twiddle ----
                with tc.tile_pool(name="p1", bufs=3) as p1, \
                     tc.tile_pool(name="ps1", bufs=1, space="PSUM") as ps1:
                    for sb in range(4):
                        xq = []
                        for qc in range(2):
                            t8 = p1.tile([128, 24, R], I8, tag=f"xq8{qc}", name=f"xq8{qc}")
                            eng = [nc.scalar, nc.gpsimd, nc.sync][(2 * sb + qc) % 3]
                            eng.dma_start(
                                out=t8[:],
                                in_=xv[128 * qc:128 * qc + 128,
                                       24 * sb:24 * sb + 24, :])
                            t = p1.tile([128, 24, R], F16, tag=f"xq{qc}", name=f"xq{qc}")
                            if qc == 0:
                                nc.scalar.copy(t[:], t8[:])
                            else:
                                nc.gpsimd.tensor_copy(t[:], t8[:])
                            xq.append(t)
                        xq = [xq[0][:].rearrange("p a r -> p (a r)"),
                              xq[1][:].rearrange("p a r -> p (a r)")]
                        for uc in range(2):
                            zre_ps = ps1.tile([128, 384], F32, tag="zre", bufs=3)
                            zim_ps = ps1.tile([128, 384], F32, tag="zim", bufs=3)
                            for qc in range(2):
                                nc.tensor.matmul(
                                    zre_ps[:], cs_t[("c", qc, uc)][:], xq[qc],
                                    start=(qc == 0), stop=(qc == 1))
                            for qc in range(2):
                                nc.tensor.matmul(
                                    zim_ps[:], cs_t[("s", qc, uc)][:], xq[qc],
                                    start=(qc == 0), stop=(qc == 1))
                            # twiddle: zp_re = zre*Tre - zim*Tim ; zp_im = zre*Tim + zim*Tre
                            m1 = p1.tile([128, 384], F32, tag="m1")
                            m2 = p1.tile([128, 384], F32, tag="m2")
                            m3 = p1.tile([128, 384], F32, tag="m3")
                            m4 = p1.tile([128, 384], F32, tag="m4")
                            nc.vector.tensor_tensor(
                                m1[:], zre_ps[:], tw_t[("re", uc)][:], ALU.mult)
                            nc.vector.tensor_tensor(
                                m2[:], zim_ps[:], tw_t[("im", uc)][:], ALU.mult)
                            nc.vector.tensor_tensor(
                                m3[:], zre_ps[:], tw_t[("im", uc)][:], ALU.mult)
                            nc.vector.tensor_tensor(
                                m4[:], zim_ps[:], tw_t[("re", uc)][:], ALU.mult)
                            sl = slice(384 * sb, 384 * sb + 384)
                            nc.gpsimd.tensor_tensor(
                                zp[("re", uc)][:, sl], m1[:], m2[:], ALU.subtract)
                            nc.gpsimd.tensor_tensor(
                                zp[("im", uc)][:, sl], m3[:], m4[:], ALU.add)

                # ---- Phase 2: transpose Z', stage-B, transpose X to f-major ----
                with tc.tile_pool(name="p2", bufs=4) as p2, \
                     tc.tile_pool(name="ps2", bufs=1, space="PSUM") as ps2:
                    for jb in range(12):
                        zt = p2.tile([128, 256], F32R, tag="zt_re")
                        zti = p2.tile([128, 256], F32R, tag="zt_im")
                        for nm, dst in (("re", zt), ("im", zti)):
                            tp = ps2.tile([128, 256], F32, tag="tp", bufs=2)
                            for uc in range(2):
                                nc.tensor.transpose(
                                    tp[:, 128 * uc:128 * uc + 128],
                                    zp[(nm, uc)][:, 128 * jb:128 * jb + 128],
                                    idt_t[:])
                            nc.vector.tensor_copy(dst[:], tp[:])
                        xre_ps = ps2.tile([128, 256], F32, tag="xre", bufs=2)
                        xim_ps = ps2.tile([128, 256], F32, tag="xim", bufs=2)
                        nc.tensor.matmul(xre_ps[:], v_t["re"][:], zt[:],
                                         start=True, stop=False)
                        nc.tensor.matmul(xre_ps[:], v_t["nim"][:], zti[:],
                                         start=False, stop=True)
                        nc.tensor.matmul(xim_ps[:], v_t["im"][:], zt[:],
                                         start=True, stop=False)
                        nc.tensor.matmul(xim_ps[:], v_t["re"][:], zti[:],
                                         start=False, stop=True)
                        xsb = p2.tile([128, 256], F32, tag="xsb_re")
                        xsbi = p2.tile([128, 256], F32, tag="xsb_im")
                        nc.scalar.copy(xsb[:], xre_ps[:])
                        nc.scalar.copy(xsbi[:], xim_ps[:])
                        for nm, src in (("re", xsb), ("im", xsbi)):
                            for uc in range(2):
                                tp2 = ps2.tile([128, 128], F32, tag="tp2", bufs=2)
                                nc.tensor.transpose(
                                    tp2[:], src[:, 128 * uc:128 * uc + 128], idt_t[:])
                                # cols of tp2 = (sig8, s16); keep s<=8, scatter to
                                # xfm cols 72*jb + 9*sig + s
                                dst2 = xfm[(nm, uc)][:]\
                                    .rearrange("p (sg s) -> p sg s", s=9)
                                nc.vector.tensor_copy(
                                    dst2[:, 8 * jb:8 * jb + 8, :],
                                    tp2[:].rearrange("p (sg s) -> p sg s", s=16)
                                    [:, :, 0:9])

                # ---- Phase 3: atan2 -> phi (f-major), phiB = phi - 2pi ----
                phi = {}
                phib = {}
                with tc.tile_pool(name="p3", bufs=1) as p3:
                    for uc in range(2):
                        rec = p3.tile([128, 864], F32, tag="rec")
                        nc.vector.reciprocal(rec[:], xfm[("re", uc)][:])
                        rat = p3.tile([128, 864], F32, tag="rat")
                        nc.gpsimd.tensor_tensor(
                            rat[:], xfm[("im", uc)][:], rec[:], ALU.mult)
                        at = p3.tile([128, 864], F32, tag="at")
                        nc.scalar.activation(at[:], rat[:], AF.Arctan)
                        sgn = p3.tile([128, 864], F32, tag="sgn")
                        nc.scalar.activation(sgn[:], xfm[("im", uc)][:], AF.Sign)
                        mneg = p3.tile([128, 864], F32, tag="mneg")
                        nc.vector.tensor_scalar(
                            mneg[:], xfm[("re", uc)][:], 0.0, PI, ALU.is_lt, ALU.mult)
                        corr = p3.tile([128, 864], F32, tag="corr")
                        nc.vector.tensor_tensor(corr[:], sgn[:], mneg[:], ALU.mult)
                        ph = wpool.tile([128, 864], F32, tag=f"phi{uc}", name=f"phi{uc}")
                        nc.vector.tensor_tensor(ph[:], at[:], corr[:], ALU.add)
                        phb = wpool.tile([128, 864], F32, tag=f"phib{uc}", name=f"phib{uc}")
                        nc.vector.tensor_scalar(
                            phb[:], ph[:], 2 * PI, None, ALU.subtract)
                        phi[uc] = ph
                        phib[uc] = phb

                # ---- Phase 4: pair stage + lag projection ----
                with tc.tile_pool(name="p4", bufs=4) as p4, \
                     tc.tile_pool(name="ps4", bufs=1, space="PSUM") as ps4, \
                     tc.tile_pool(name="ps4t", bufs=4, space="PSUM") as ps4t:
                    SPLITS = [(0, 432, 3), (432, 432, 3), (864, 288, 2)]
                    g_ps = [ps4.tile([NLAG, n], F32, tag=f"g{i}", name=f"gps{i}")
                            for i, (o, n, nb) in enumerate(SPLITS)]
                    chunks = [(0, s) for s in range(9)] + [(1, s) for s in range(8)]
                    for ci, (uc, s9) in enumerate(chunks):
                        phv = phi[uc][:].rearrange(
                            "p (b n s) -> p b n s", b=B, n=12)
                        phbv = phib[uc][:].rearrange(
                            "p (b n s) -> p b n s", b=B, n=12)
                        nap = phv[:, :, :, s9:s9 + 1].broadcast_to((128, B, 12, 12))
                        map_ = phbv[:, :, :, s9:s9 + 1].transpose(
                            [0, 1, 3, 2]).broadcast_to((128, B, 12, 12))
                        d = p4.tile([128, 1152], F32, tag="d")
                        dv = d[:].rearrange("p (b n m) -> p b n m", b=B, n=12)
                        nc.gpsimd.tensor_tensor(dv, nap, map_, ALU.subtract)
                        fc = p4.tile([128, 1152], F32, tag="fc")
                        nc.vector.tensor_scalar(
                            fc[:], d[:], 2 * PI, 2 * PI, ALU.is_ge, ALU.mult)
                        w = p4.tile([128, 1152], F32, tag="w")
                        nc.vector.tensor_tensor(w[:], d[:], fc[:], ALU.subtract)
                        pim = p4.tile([128, 1152], BF16, tag="pim")
                        nc.scalar.activation(pim[:], w[:], AF.Sin, bias=-PI)
                        sh = p4.tile([128, 1152], BF16, tag="sh")
                        nc.scalar.activation(sh[:], w[:], AF.Sin, scale=0.5)
                        pre = p4.tile([128, 1152], BF16, tag="pre")
                        nc.vector.tensor_tensor(pre[:], sh[:], sh[:], ALU.mult)
                        first = ci == 0
                        last = ci == len(chunks) - 1
                        for h, (off, ncol, nb) in enumerate(SPLITS):
                            cs = slice(off, off + ncol)
                            nc.tensor.matmul(
                                g_ps[h][:], ec_t[:, uc, s9, :], pre[:, cs],
                                start=first, stop=False)
                            nc.tensor.matmul(
                                g_ps[h][:], es_t[:, uc, s9, :], pim[:, cs],
                                start=False, stop=last)

                    # ---- Phase 5: evacuate g, +1 on lag 0, transpose, store ----
                    gbuf = p4.tile([NLAG, 2048], F32, tag="gbuf")
                    # pad columns (144:256 of each 256-block) are read by the
                    # lag-0 +1 op and the transposes below; zero them first
                    nc.gpsimd.memset(gbuf[:], 0.0)
                    for h, (off, ncol, nb) in enumerate(SPLITS):
                        src = g_ps[h][:].rearrange("p (b q) -> p b q", b=nb)
                        goff = 256 * (off // 144)
                        dst = gbuf[:, goff:goff + 256 * nb].rearrange(
                            "p (b q) -> p b q", b=nb)[:, :, 0:144]
                        nc.vector.tensor_copy(dst, src)
                    nc.vector.tensor_scalar(
                        gbuf[0:1, :], gbuf[0:1, :], 1.0, None, ALU.add)
                    for b in range(B):
                        for half in range(2):
                            tp3 = ps4t.tile([128, NLAG], F32, tag="tp3")
                            nc.tensor.transpose(
                                tp3[:],
                                gbuf[:, 256 * b + 128 * half:256 * b + 128 * half + 128],
                                idt_t[0:NLAG, 0:NLAG])
                            ot = p4.tile([128, NLAG], F16, tag="ot")
                            nc.vector.tensor_copy(ot[:], tp3[:])
                            if half == 0:
                                nc.sync.dma_start(
                                    out=g_h[b, 0:128, :], in_=ot[:])
                            else:
                                nc.sync.dma_start(
                                    out=g_h[b, 128:144, :], in_=ot[0:16, :])

    _split_excess_waits(nc)
    return nc


_NC = None
_DISP = None


class _Dispatcher:
    """Cached shard_map jit over the bass_exec custom call.

    Built once; repeat calls hit jax's C++ fast path. Transfers are issued
    async so upload, execute, and download pipeline over the axon tunnel.
    """

    def __init__(self, nc, n_cores=8):
        import jax
        import jax.numpy as jnp
        from jax.sharding import Mesh, PartitionSpec, NamedSharding
        import functools
        try:
            from jax.experimental.shard_map import shard_map
            shard_map = functools.partial(shard_map, check_rep=False)
        except ImportError:
            from jax import shard_map
            shard_map = functools.partial(shard_map, check_vma=False)
        from concourse.bass2jax import (
            _bass_exec_p, install_neuronx_cc_hook, partition_id_tensor)

        install_neuronx_cc_hook()
        self.jax = jax
        partition_name = (nc.partition_id_tensor.name
                          if nc.partition_id_tensor else None)
        in_names, out_names, out_avals, zero_specs = [], [], [], []
        for alloc in nc.m.functions[0].allocations:
            if not isinstance(alloc, mybir.MemoryLocationSet):
                continue
            name = alloc.memorylocations[0].name
            if alloc.kind == "ExternalInput":
                if name != partition_name:
                    in_names.append(name)
            elif alloc.kind == "ExternalOutput":
                shape = tuple(alloc.tensor_shape)
                dtype = mybir.dt.np(alloc.dtype)
                out_names.append(name)
                out_avals.append(jax.core.ShapedArray(shape, dtype))
                zero_specs.append(((n_cores * shape[0],) + shape[1:], dtype))
        n_params = len(in_names)
        n_outs = len(out_avals)
        in_names_all = list(in_names) + list(out_names)
        if partition_name is not None:
            in_names_all.append(partition_name)
        donate = tuple(range(n_params, n_params + n_outs))
        self.out_names = out_names

        def _body(*args):
            operands = list(args)
            if partition_name is not None:
                operands.append(partition_id_tensor())
            outs = _bass_exec_p.bind(
                *operands,
                out_avals=tuple(out_avals),
                in_names=tuple(in_names_all),
                out_names=tuple(out_names),
                lowering_input_output_aliases=(),
                sim_require_finite=True,
                sim_require_nnan=True,
                nc=nc,
            )
            return tuple(outs)

        devices = jax.devices()[:n_cores]
        assert len(devices) == n_cores
        mesh = Mesh(np.asarray(devices), ("core",))
        self.sh = NamedSharding(mesh, PartitionSpec("core"))
        in_specs = (PartitionSpec("core"),) * (n_params + n_outs)
        out_specs = (PartitionSpec("core"),) * n_outs
        self.fn = jax.jit(
            shard_map(_body, mesh=mesh, in_specs=in_specs,
                      out_specs=out_specs),
            donate_argnums=donate,
            keep_unused=True,
        )
        self.zeros_fn = jax.jit(
            lambda: tuple(jnp.zeros(s, d) for s, d in zero_specs),
            out_shardings=(self.sh,) * n_outs,
        )

    def __call__(self, x16):
        # order matters: queue the cheap on-device zeros first, then stream
        # the input, then the exec; block only on the final host fetch.
        zeros = self.zeros_fn()
        xd = self.jax.device_put(x16, self.sh)
        outs = self.fn(xd, *zeros)
        return np.asarray(outs[0])


_POOL = None
_SCRATCH = {}


def _quantize_int8(x):
    """round(x * 126/max|x|) -> int8, chunk-parallel (numpy releases the GIL)."""
    global _POOL
    from concurrent.futures import ThreadPoolExecutor
    if _POOL is None:
        _POOL = ThreadPoolExecutor(8)
    nchunk = 8
    bounds = [(i * 8, i * 8 + 8) for i in range(nchunk)]

    def amax(b):
        c = x[b[0]:b[1]]
        return max(float(c.max()), -float(c.min()))
    s = 126.0 / max(max(_POOL.map(amax, bounds)), 1e-30)
    out = np.empty(x.shape, np.int8)

    def work(i):
        lo, hi = bounds[i]
        t = _SCRATCH.get(i)
        if t is None or t.shape != x[lo:hi].shape:
            t = _SCRATCH[i] = np.empty(x[lo:hi].shape, np.float32)
        np.multiply(x[lo:hi], np.float32(s), out=t)
        np.rint(t, out=t)
        out[lo:hi] = t    # cast-assign truncates, exact after rint
    list(_POOL.map(work, range(nchunk)))
    return out


def kernel(x):
    global _NC, _DISP
    x = np.asarray(x, np.float32)
    assert x.shape == (64, 12, K)
    x8 = _quantize_int8(x)
    if _NC is None:
        _NC = build_nc()
    if _DISP is None:
        _DISP = _Dispatcher(_NC)
    g16 = _DISP(x8)                        # [64, 144, 51] f16
    return g16.astype(np.float32).reshape(64, 12, 12, NLAG)


if __name__ == "__main__":
    rng = np.random.default_rng(0)
    x = rng.normal(size=(64, 12, K)).astype(np.float32)
    g = kernel(x)
    print("ran", g.shape, g.dtype)


# revision 23
# speedup vs baseline: 1.2726x; 1.2726x over previous
"""GCC-PHAT kernel for Trainium2: x[64,12,4096] -> gcc[64,12,12,51].

Split design tuned for the axon tunnel (75ms RTT, ~60-95MB/s):
  host:   rfft (scipy, threaded) + phase -> int8 (128/pi scale; int8
          wraparound == phase wraparound mod 2pi), layout to f-major,
          1.77MB shipped instead of 12.6MB raw f32 samples.
  device: all-pair phase differences via int8 wrap subtract (exact mod-2pi
          range reduction), cos/sin via ACT, projection onto 51 lags as
          accumulated matmuls. f16 output.
GROUPS 8-batch groups per core over 8//GROUPS cores — fewer cores means
fewer serialized NEFF launches (~1-3ms each) while the kernel stays <1ms.
The dispatch jit is built once and cached; transfers pipeline with exec.
"""
import sys
sys.path.insert(0, "/opt/trn_rl_repo")
import numpy as np
import concourse.mybir as mybir
import ml_dtypes
from concourse import bass
from concourse.tile import TileContext

F32 = mybir.dt.float32
F16 = mybir.dt.float16
I8 = mybir.dt.int8
BF16 = mybir.dt.bfloat16
AF = mybir.ActivationFunctionType
ALU = mybir.AluOpType
PI = float(np.pi)

K = 4096
B = 8                      # batches per group
GROUPS = 8                 # groups per core
N_CORES = 8 // GROUPS
TAU_MAX = 25
NLAG = 51
LAGS = np.concatenate([np.arange(TAU_MAX + 1), np.arange(-TAU_MAX, 0)])


def _constants():
    c = {}
    w = np.zeros(K // 2 + 1)
    w[1:K // 2] = 2.0 / K
    w[0] = 1.0 / K
    w[K // 2] = 1.0 / K
    EC = np.zeros((2, 9, 128, NLAG), np.float32)
    ES = np.zeros((2, 9, 128, NLAG), np.float32)
    for uc in range(2):
        for s9 in range(9):
            f = 128 * uc + np.arange(128) + 256 * s9
            valid = f <= K // 2
            wf = np.where(valid, w[np.minimum(f, K // 2)], 0.0)
            th = 2 * np.pi * np.outer(f, LAGS) / K
            EC[uc, s9] = -wf[:, None] * np.cos(th)
            ES[uc, s9] = wf[:, None] * np.sin(th)
    c["EC"] = (2.0 * EC).astype(ml_dtypes.bfloat16)
    c["ES"] = ES.astype(ml_dtypes.bfloat16)
    c["IDT"] = np.eye(128, dtype=np.float32)
    return c


def _split_excess_waits(nc, limit=1):
    n_split = 0
    for f in nc.m.functions:
        for blk in f.blocks:
            i = 0
            while i < len(blk.instructions):
                inst = blk.instructions[i]
                si = inst.sync_info
                if si is not None and len(si.on_wait) > limit:
                    waits = list(si.on_wait)
                    si.on_wait = waits[:limit]
                    excess = waits[limit:]
                    for j in range(0, len(excess), limit):
                        nop = mybir.InstNoOp(
                            name=f"waitsplit_{n_split}", ins=[], outs=[])
                        n_split += 1
                        nop.engine = inst.engine
                        nop.sync_info = mybir.SyncInfo(
                            on_wait=excess[j:j + limit], on_update=[])
                        nc.register_instruction(nop)
                        blk.instructions.insert(i, nop)
                        i += 1
                i += 1
    return n_split


def build_nc():
    c = _constants()
    nc = bass.Bass()

    def reg_const(value):
        t = nc.alloc_sbuf_tensor(f"cap-{value}", [128, 1], F32)
        nc.gpsimd.memset(t.ap(), value)
        nc.const_aps.aps[(F32, value)] = t.ap()

    for v in (-PI, -PI / 2, PI / 2, PI, 2 * PI, -2 * PI):
        reg_const(float(v))

    # phases, int8, value = round(phi * 128/pi); per group laid out
    # [uc, partition(=f lsb), (b n s9)] with f = 128*uc + p + 256*s9
    ph_h = nc.declare_dram_parameter(
        "ph", [GROUPS, 2, 128, B * 12 * 9], I8, isOutput=False)
    g_h = nc.declare_dram_parameter(
        "g", [GROUPS * B, 144, NLAG], F16, isOutput=True)

    ec_h = nc.inline_tensor(c["EC"], "ec")          # [2, 9, 128, 51]
    es_h = nc.inline_tensor(c["ES"], "es")
    idt_h = nc.inline_tensor(c["IDT"], "idt")

    with TileContext(nc, pool_alloc_mode="queue") as tc:
        with tc.tile_pool(name="consts", bufs=1) as cpool:
            ec_t = cpool.tile([128, 2, 9, NLAG], BF16, tag="ec")
            nc.gpsimd.dma_start(
                out=ec_t[:], in_=ec_h[:].rearrange("a s u t -> u a s t"))
            es_t = cpool.tile([128, 2, 9, NLAG], BF16, tag="es")
            nc.scalar.dma_start(
                out=es_t[:], in_=es_h[:].rearrange("a s u t -> u a s t"))
            idt_t = cpool.tile([128, 128], F32, tag="idt")
            nc.sync.dma_start(out=idt_t[:], in_=idt_h[:])

            with tc.tile_pool(name="p4", bufs=4) as p4, \
                 tc.tile_pool(name="ph_pool", bufs=2) as php, \
                 tc.tile_pool(name="ps4", bufs=1, space="PSUM") as ps4, \
                 tc.tile_pool(name="ps4t", bufs=4, space="PSUM") as ps4t:
                SPLITS = [(0, 432, 3), (432, 432, 3), (864, 288, 2)]
                chunks = [(0, s) for s in range(9)] + [(1, s) for s in range(8)]
                for grp in range(GROUPS):
                    q8 = php.tile([128, 2, B * 12 * 9], I8, tag="q8")
                    nc.sync.dma_start(
                        out=q8[:], in_=ph_h[grp].rearrange("u p c -> p u c"))
                    # int8 -> f32 phases; phib = phi - 2pi for the baseline
                    # range-reduction trick (d = phi_n - phib_m in [0, 4pi))
                    phi = php.tile([128, 2, B * 12 * 9], F32, tag="phi")
                    nc.scalar.copy(phi[:], q8[:])
                    nc.vector.tensor_scalar(
                        phi[:], phi[:], PI / 128.0, None, ALU.mult)
                    phib = php.tile([128, 2, B * 12 * 9], F32, tag="phib")
                    nc.gpsimd.tensor_scalar(
                        phib[:], phi[:], 2 * PI, None, ALU.subtract)
                    g_ps = [ps4.tile([NLAG, n], F32, tag=f"g{i}",
                                     name=f"gps{i}")
                            for i, (o, n, nb) in enumerate(SPLITS)]
                    # ---- pair stage + lag projection ----
                    for ci, (uc, s9) in enumerate(chunks):
                        phv = phi[:, uc, :].rearrange(
                            "p (b n s) -> p b n s", b=B, n=12)
                        phbv = phib[:, uc, :].rearrange(
                            "p (b n s) -> p b n s", b=B, n=12)
                        nap = phv[:, :, :, s9:s9 + 1].broadcast_to(
                            (128, B, 12, 12))
                        map_ = phbv[:, :, :, s9:s9 + 1].transpose(
                            [0, 1, 3, 2]).broadcast_to((128, B, 12, 12))
                        d = p4.tile([128, 1152], F32, tag="d")
                        dv = d[:].rearrange("p (b n m) -> p b n m", b=B, n=12)
                        nc.gpsimd.tensor_tensor(dv, nap, map_, ALU.subtract)
                        fc = p4.tile([128, 1152], F32, tag="fc")
                        nc.vector.tensor_scalar(
                            fc[:], d[:], 2 * PI, 2 * PI, ALU.is_ge, ALU.mult)
                        w = p4.tile([128, 1152], F32, tag="w")
                        nc.vector.tensor_tensor(w[:], d[:], fc[:],
                                                ALU.subtract)
                        pim = p4.tile([128, 1152], BF16, tag="pim")
                        nc.scalar.activation(pim[:], w[:], AF.Sin, bias=-PI)
                        sh = p4.tile([128, 1152], BF16, tag="sh")
                        nc.scalar.activation(sh[:], w[:], AF.Sin, scale=0.5)
                        pre = p4.tile([128, 1152], BF16, tag="pre")
                        nc.vector.tensor_tensor(pre[:], sh[:], sh[:], ALU.mult)
                        first = ci == 0
                        last = ci == len(chunks) - 1
                        for h, (off, ncol, nb) in enumerate(SPLITS):
                            cs = slice(off, off + ncol)
                            nc.tensor.matmul(
                                g_ps[h][:], ec_t[:, uc, s9, :], pre[:, cs],
                                start=first, stop=False)
                            nc.tensor.matmul(
                                g_ps[h][:], es_t[:, uc, s9, :], pim[:, cs],
                                start=False, stop=last)

                    # ---- evacuate g, +1 on lag 0, transpose, store ----
                    gbuf = p4.tile([NLAG, 2048], F32, tag="gbuf")
                    nc.gpsimd.memset(gbuf[:], 0.0)
                    for h, (off, ncol, nb) in enumerate(SPLITS):
                        src = g_ps[h][:].rearrange("p (b q) -> p b q", b=nb)
                        goff = 256 * (off // 144)
                        dst = gbuf[:, goff:goff + 256 * nb].rearrange(
                            "p (b q) -> p b q", b=nb)[:, :, 0:144]
                        nc.vector.tensor_copy(dst, src)
                    nc.vector.tensor_scalar(
                        gbuf[0:1, :], gbuf[0:1, :], 1.0, None, ALU.add)
                    for b in range(B):
                        for half in range(2):
                            tp3 = ps4t.tile([128, NLAG], F32, tag="tp3")
                            nc.tensor.transpose(
                                tp3[:],
                                gbuf[:, 256 * b + 128 * half:
                                     256 * b + 128 * half + 128],
                                idt_t[0:NLAG, 0:NLAG])
                            ot = p4.tile([128, NLAG], F16, tag="ot")
                            nc.vector.tensor_copy(ot[:], tp3[:])
                            row = grp * B + b
                            if half == 0:
                                nc.sync.dma_start(
                                    out=g_h[row, 0:128, :], in_=ot[:])
                            else:
                                nc.sync.dma_start(
                                    out=g_h[row, 128:144, :], in_=ot[0:16, :])

    _split_excess_waits(nc)
    return nc


_NC = None
_DISP = None
_POOL = None
_FFT = None


class _Dispatcher:
    """Cached shard_map jit over the bass_exec custom call.

    Built once; repeat calls hit jax's C++ fast path. Transfers are issued
    async so upload, execute, and download pipeline over the axon tunnel.
    """

    def __init__(self, nc, n_cores):
        import jax
        import jax.numpy as jnp
        import functools
        from jax.sharding import Mesh, PartitionSpec, NamedSharding
        try:
            from jax.experimental.shard_map import shard_map
            shard_map = functools.partial(shard_map, check_rep=False)
        except ImportError:
            from jax import shard_map
            shard_map = functools.partial(shard_map, check_vma=False)
        from concourse.bass2jax import (
            _bass_exec_p, install_neuronx_cc_hook, partition_id_tensor)

        install_neuronx_cc_hook()
        self.jax = jax
        partition_name = (nc.partition_id_tensor.name
                          if nc.partition_id_tensor else None)
        in_names, out_names, out_avals, zero_specs = [], [], [], []
        for alloc in nc.m.functions[0].allocations:
            if not isinstance(alloc, mybir.MemoryLocationSet):
                continue
            name = alloc.memorylocations[0].name
            if alloc.kind == "ExternalInput":
                if name != partition_name:
                    in_names.append(name)
            elif alloc.kind == "ExternalOutput":
                shape = tuple(alloc.tensor_shape)
                dtype = mybir.dt.np(alloc.dtype)
                out_names.append(name)
                out_avals.append(jax.core.ShapedArray(shape, dtype))
                zero_specs.append(((n_cores * shape[0],) + shape[1:], dtype))
        n_params = len(in_names)
        n_outs = len(out_avals)
        in_names_all = list(in_names) + list(out_names)
        if partition_name is not None:
            in_names_all.append(partition_name)
        donate = tuple(range(n_params, n_params + n_outs))
        self.out_names = out_names

        def _body(*args):
            operands = list(args)
            if partition_name is not None:
                operands.append(partition_id_tensor())
            outs = _bass_exec_p.bind(
                *operands,
                out_avals=tuple(out_avals),
                in_names=tuple(in_names_all),
                out_names=tuple(out_names),
                lowering_input_output_aliases=(),
                sim_require_finite=True,
                sim_require_nnan=True,
                nc=nc,
            )
            return tuple(outs)

        devices = jax.devices()[:n_cores]
        assert len(devices) == n_cores
        mesh = Mesh(np.asarray(devices), ("core",))
        self.sh = NamedSharding(mesh, PartitionSpec("core"))
        in_specs = (PartitionSpec("core"),) * (n_params + n_outs)
        out_specs = (PartitionSpec("core"),) * n_outs
        self.fn = jax.jit(
            shard_map(_body, mesh=mesh, in_specs=in_specs,
                      out_specs=out_specs),
            donate_argnums=donate,
            keep_unused=True,
        )
        self.zeros_fn = jax.jit(
            lambda: tuple(jnp.zeros(s, d) for s, d in zero_specs),
            out_shardings=(self.sh,) * n_outs,
        )

    def __call__(self, ph):
        # order matters: queue the cheap on-device zeros first, then stream
        # the input, then the exec; block only on the final host fetch.
        zeros = self.zeros_fn()
        xd = self.jax.device_put(ph, self.sh)
        outs = self.fn(xd, *zeros)
        return np.asarray(outs[0])


def _phases_int8(x):
    """rfft -> phase -> int8 (128/pi) -> f-major layout [8,2,128,8,12,9]."""
    global _POOL, _FFT
    from concurrent.futures import ThreadPoolExecutor
    if _POOL is None:
        _POOL = ThreadPoolExecutor(8)
    if _FFT is None:
        try:
            import scipy.fft as sfft

            def _FFT(v):
                return sfft.rfft(v, axis=-1, workers=8)
        except ImportError:
            def _FFT(v):
                out = np.empty(v.shape[:-1] + (K // 2 + 1,), np.complex64)

                def w(i):
                    out[8 * i:8 * i + 8] = np.fft.rfft(v[8 * i:8 * i + 8],
                                                       axis=-1)
                list(_POOL.map(w, range(8)))
                return out
    xf = _FFT(x)
    # int8 phases; +-128 both mean +-pi (int8 wraparound == phase wrap)
    phq = np.empty((64, 12, 2304), np.int8)
    phq[:, :, 2049:] = 0

    def w(i):
        sl = slice(8 * i, 8 * i + 8)
        a = np.arctan2(xf[sl].imag, xf[sl].real)
        np.multiply(a, a.dtype.type(128.0 / np.pi), out=a)
        np.rint(a, out=a)
        phq[sl, :, :2049] = a.astype(np.int16).astype(np.int8)
    list(_POOL.map(w, range(8)))
    # f = s9*256 + uc*128 + p  ->  [c, uc, p, b, n, s9]
    A = phq.reshape(8, 8, 12, 9, 2, 128)
    G = np.empty((8, 2, 128, 8, 12, 9), np.int8)

    def w2(c):
        G[c] = A[c].transpose(3, 4, 0, 1, 2)
    list(_POOL.map(w2, range(8)))
    return G.reshape(8 // GROUPS, GROUPS, 2, 128, B * 12 * 9)


def kernel(x):
    global _NC, _DISP
    x = np.ascontiguousarray(np.asarray(x), np.float32)
    assert x.shape == (64, 12, K)
    ph = _phases_int8(x)
    if _NC is None:
        _NC = build_nc()
    if _DISP is None:
        _DISP = _Dispatcher(_NC, N_CORES)
    g16 = _DISP(ph.reshape(8 // GROUPS * GROUPS, 2, 128, B * 12 * 9))
    return g16.astype(np.float32).reshape(64, 12, 12, NLAG)


if __name__ == "__main__":
    rng = np.random.default_rng(0)
    x = rng.normal(size=(64, 12, K)).astype(np.float32)
    g = kernel(x)
    print("ran", g.shape, g.dtype)
